# revision 19
# baseline (speedup 1.0000x reference)
"""GravityField Trainium2 kernel.

out = U * sqrt(1 + clip(0.1 * grav, -0.9, 5) + 1e-6)
where grav[t] = phi[t] . sum_t'(phi[t'] * mass[t']), phi = sqrt(2/R)*cos(coords@W+b),
mass = softplus(relu(coords@w1+b1)@w2+b2).

Sharding: pure data-parallel over B (8 batches -> 8 cores, no communication).
Each core processes coords [8192, 64] and U [8192, 512] (= 64*8 flattened).

The Scalar Engine Sin only accepts [-pi, pi], so the RFF argument (std ~16)
is range-reduced with the fp32 magic-rounding trick before the activation.
"""

import sys

sys.path.insert(0, "/opt/trn_rl_repo")

import numpy as np
from contextlib import ExitStack

import concourse.bass as bass
import concourse.bacc as bacc
import concourse.mybir as mybir
from concourse import tile
from concourse.bass_utils import run_bass_kernel_spmd
from concourse.masks import make_identity

F32 = mybir.dt.float32
AF = mybir.ActivationFunctionType
ALU = mybir.AluOpType

B, T, D, R_LR, N_RFF = 8, 8192, 64, 8, 64
F = D * R_LR  # 512 floats of U per (b, t)
STRENGTH = 0.1
HALF_PI = 1.5707963267948966
TWO_PI = 6.283185307179586
INV_2PI = 0.15915494309189535
MAGIC = 12582912.0  # 1.5 * 2**23: fp32 add/sub rounds to nearest integer
PI_CLAMP = 3.14159  # strictly inside [-pi, pi] for the ACT Sin table
BIGC = 512  # pass-1 chunk (T rows)
N_BIG = T // BIGC  # 16
CHUNK = 128  # pass-2 chunk (T rows)
N_CHUNKS = T // CHUNK  # 64
# grav_true = (2/N_RFF) * rawgrav ; influence = STRENGTH * grav_true
PHI_SUM_SCALE = STRENGTH * 2.0 / N_RFF


def build_program():
    nc = bacc.Bacc("TRN2", target_bir_lowering=False, debug=False, num_devices=8)

    u_d = nc.dram_tensor("U", [T, F], F32, kind="ExternalInput")
    coords_d = nc.dram_tensor("coords", [T, D], F32, kind="ExternalInput")
    w1_d = nc.dram_tensor("mass_w1", [D, D], F32, kind="ExternalInput")
    b1_d = nc.dram_tensor("mass_b1", [D], F32, kind="ExternalInput")
    w2_d = nc.dram_tensor("mass_w2", [D, 1], F32, kind="ExternalInput")
    b2_d = nc.dram_tensor("mass_b2", [1], F32, kind="ExternalInput")
    rffw_d = nc.dram_tensor("rff_W", [D, N_RFF], F32, kind="ExternalInput")
    rffb_d = nc.dram_tensor("rff_b", [N_RFF], F32, kind="ExternalInput")
    out_d = nc.dram_tensor("out", [T, F], F32, kind="ExternalOutput")

    with tile.TileContext(nc) as tc, ExitStack() as ctx:
        const = ctx.enter_context(tc.tile_pool(name="const", bufs=1))

        identity = const.tile([128, 128], F32)
        make_identity(nc, identity[:])

        # lhsT for the fused h/phi matmul: [65, 128]
        #   cols 0:64  -> mass_w1 (rows 0:64) + mass_b1 (row 64)
        #   cols 64:128-> rff_W   (rows 0:64) + rff_b + pi/2 (row 64)
        # row 64 multiplies the ones-row of the augmented coords^T -> bias add;
        # +pi/2 turns Sin into cos.
        # Staged: matmul stationary operands must have a single producing
        # engine (the PE LW micro-op encodes very few semaphore waits), so
        # the DMA-written staging tile is bounced through one DVE copy.
        w_stage = const.tile([65, 128], F32)
        nc.sync.dma_start(w_stage[0:64, 0:64], w1_d[:, :])
        nc.sync.dma_start(w_stage[64:65, 0:64], b1_d[None, :])
        nc.sync.dma_start(w_stage[0:64, 64:128], rffw_d[:, :])
        nc.sync.dma_start(w_stage[64:65, 64:128], rffb_d[None, :])
        nc.vector.tensor_scalar_add(w_stage[64:65, 64:128], w_stage[64:65, 64:128], HALF_PI)
        w_comb = const.tile([65, 128], F32)
        nc.vector.tensor_copy(w_comb[:], w_stage[:])
        w2_sb = const.tile([D, 1], F32)
        nc.sync.dma_start(w2_sb[:], w2_d[:, :])
        ones_row = const.tile([1, 128], F32)
        nc.vector.memset(ones_row[:], 1.0)
        b2_sb = const.tile([1, 1], F32)
        nc.sync.dma_start(b2_sb[:], b2_d[None, :])
        b2_neg_sb = const.tile([1, 1], F32)
        nc.vector.tensor_scalar_mul(b2_neg_sb[:], b2_sb[:], -1.0)
        b2_neg = const.tile([128, 1], F32)
        sqrt_bias = const.tile([128, 1], F32)
        nc.vector.memset(sqrt_bias[:], 1.000001)
        phi_sum = const.tile([N_RFF, 1], F32)

        # cos features for all T, [R, T] layout, kept resident for pass 2
        phiT_all = const.tile([N_RFF, T], F32)

        coords_pool = ctx.enter_context(tc.tile_pool(name="coords", bufs=3))
        caug_pool = ctx.enter_context(tc.tile_pool(name="caug", bufs=2))
        hT_pool = ctx.enter_context(tc.tile_pool(name="hT", bufs=2))
        rr_pool = ctx.enter_context(tc.tile_pool(name="rr", bufs=2))
        phiB_pool = ctx.enter_context(tc.tile_pool(name="phiB", bufs=3))
        mass_pool = ctx.enter_context(tc.tile_pool(name="mass", bufs=2))
        u_pool = ctx.enter_context(tc.tile_pool(name="u", bufs=N_CHUNKS))
        scale_pool = ctx.enter_context(tc.tile_pool(name="scale", bufs=4))

        u_tiles = []

        with (
            tc.tile_pool(name="ptr", bufs=2, space=bass.MemorySpace.PSUM) as ptr_pool,
            tc.tile_pool(name="pbig", bufs=2, space=bass.MemorySpace.PSUM) as pbig_pool,
            tc.tile_pool(name="ppb", bufs=2, space=bass.MemorySpace.PSUM) as ppb_pool,
            tc.tile_pool(name="pmass", bufs=1, space=bass.MemorySpace.PSUM) as pmass_pool,
            tc.tile_pool(name="pacc", bufs=1, space=bass.MemorySpace.PSUM) as pacc_pool,
        ):
            # broadcast -b2 scalar to [128, 1] via ones-vector matmul
            pm0 = pmass_pool.tile([128, 4], F32, tag="pm")
            nc.tensor.matmul(pm0[:, 0:1], ones_row[:], b2_neg_sb[:], start=True, stop=True)
            nc.vector.tensor_copy(b2_neg[:], pm0[:, 0:1])

            acc = pacc_pool.tile([N_RFF, 1], F32)

            for c in range(N_BIG):
                tsl = slice(c * BIGC, (c + 1) * BIGC)

                # coords rows [c*512, (c+1)*512) as [128, 4*64]:
                # partition p, free block j holds row c*512 + j*128 + p
                ct = coords_pool.tile([128, 4 * D], F32, tag="ct")
                src = coords_d[tsl, :].rearrange("(j p) d -> p j d", p=128)
                # SWDGE: one semaphore per transfer, so the PE transpose
                # (whose LW micro-op has few wait slots) sees a single wait
                nc.gpsimd.dma_start(ct[:].rearrange("p (j d) -> p j d", j=4), src)

                # transpose to [64, 512] and augment with a ones row
                tp = ptr_pool.tile([D, BIGC], F32, tag="tp")
                for j in range(4):
                    nc.tensor.transpose(
                        tp[:, j * 128 : (j + 1) * 128],
                        ct[:, j * D : (j + 1) * D],
                        identity[:],
                    )
                caug = caug_pool.tile([D + 1, BIGC], F32, tag="caug")
                nc.vector.tensor_copy(caug[0:D, :], tp[:])
                nc.vector.memset(caug[D : D + 1, :], 1.0)

                # fused: rows 0:64 = (coords@w1+b1)^T, rows 64:128 = rff arg^T
                big = pbig_pool.tile([128, BIGC], F32, tag="big")
                nc.tensor.matmul(big[:], w_comb[:], caug[:], start=True, stop=True)

                hT = hT_pool.tile([D, BIGC], F32, tag="hT")
                nc.scalar.activation(hT[:], big[0:D, :], AF.Relu)

                # range-reduce x -> [-pi, pi]: y = x - 2pi*round(x/2pi)
                x = big[D : 2 * D, :]
                tmp = rr_pool.tile([D, BIGC], F32, tag="tmp")
                nc.vector.tensor_scalar(
                    tmp[:], x, INV_2PI, MAGIC, op0=ALU.mult, op1=ALU.add
                )
                nc.vector.tensor_scalar(
                    tmp[:], tmp[:], MAGIC, -TWO_PI, op0=ALU.subtract, op1=ALU.mult
                )
                nc.vector.tensor_tensor(tmp[:], x, tmp[:], op=ALU.add)
                nc.vector.tensor_scalar(
                    tmp[:], tmp[:], PI_CLAMP, -PI_CLAMP, op0=ALU.min, op1=ALU.max
                )
                nc.scalar.activation(phiT_all[:, tsl], tmp[:], AF.Sin)

                # mass[t] = softplus(h @ w2 + b2), [128, 1] per 128-subchunk
                pm = pmass_pool.tile([128, 4], F32, tag="pm")
                for j in range(4):
                    nc.tensor.matmul(
                        pm[:, j : j + 1],
                        hT[:, j * 128 : (j + 1) * 128],
                        w2_sb[:],
                        start=True,
                        stop=True,
                    )
                # -softplus(x + b2) = ln(sigmoid(-(x + b2))); the sign is
                # folded into PHI_SUM_SCALE below (acc is linear in mass)
                mass = mass_pool.tile([128, 4], F32, tag="mass")
                nc.scalar.activation(mass[:], pm[:], AF.Sigmoid, bias=b2_neg[:], scale=-1.0)
                nc.scalar.activation(mass[:], mass[:], AF.Ln)

                # phi in [T, R] layout via PE transpose of phiT, then
                # phi_sum[r] += sum_t phi[t, r] * mass[t]
                for j in range(4):
                    pb = ppb_pool.tile([128, N_RFF], F32, tag="pb")
                    nc.tensor.transpose(
                        pb[:],
                        phiT_all[:, c * BIGC + j * 128 : c * BIGC + (j + 1) * 128],
                        identity[0:N_RFF, 0:N_RFF],
                    )
                    phiB = phiB_pool.tile([128, N_RFF], F32, tag="phiB")
                    nc.scalar.copy(phiB[:], pb[:])
                    nc.tensor.matmul(
                        acc[:],
                        phiB[:],
                        mass[:, j : j + 1],
                        start=(c == 0 and j == 0),
                        stop=(c == N_BIG - 1 and j == 3),
                        skip_group_check=True,
                    )

                # prefetch U chunks while pass 1 runs
                for j in range(4):
                    usl = slice(c * BIGC + j * 128, c * BIGC + (j + 1) * 128)
                    ut = u_pool.tile([CHUNK, F], F32, tag="u")
                    nc.sync.dma_start(ut[:], u_d[usl, :])
                    u_tiles.append(ut)

            nc.scalar.mul(phi_sum[:], acc[:], -PHI_SUM_SCALE)

        with tc.tile_pool(name="pgrav", bufs=4, space=bass.MemorySpace.PSUM) as pgrav_pool:
            for c in range(N_CHUNKS):
                tsl = slice(c * CHUNK, (c + 1) * CHUNK)
                pg = pgrav_pool.tile([CHUNK, 1], F32, tag="pg")
                # influence[t] = phi[t, :] . phi_sum  (scales folded into phi_sum)
                nc.tensor.matmul(pg[:], phiT_all[:, tsl], phi_sum[:], start=True, stop=True)
                sc = scale_pool.tile([CHUNK, 1], F32, tag="sc")
                nc.vector.tensor_scalar(
                    sc[:], pg[:], -0.9, 5.0, op0=ALU.max, op1=ALU.min
                )
                nc.scalar.activation(sc[:], sc[:], AF.Sqrt, bias=sqrt_bias[:])

                ut = u_tiles[c]
                if c % 2 == 0:
                    nc.vector.tensor_scalar_mul(ut[:], ut[:], sc[:])
                else:
                    nc.scalar.mul(ut[:], ut[:], sc[:])
                nc.sync.dma_start(out_d[tsl, :], ut[:])

    nc.compile()
    return nc


_NC_CACHE = None


def _get_program():
    global _NC_CACHE
    if _NC_CACHE is None:
        _NC_CACHE = build_program()
    return _NC_CACHE


def run(inputs: dict, trace: bool = False, tmpdir=None):
    nc = _get_program()
    U = np.ascontiguousarray(np.asarray(inputs["U"], dtype=np.float32)).reshape(B, T, F)
    coords = np.ascontiguousarray(np.asarray(inputs["coords"], dtype=np.float32))
    shared = {
        "mass_w1": np.ascontiguousarray(np.asarray(inputs["mass_w1"], np.float32)),
        "mass_b1": np.ascontiguousarray(np.asarray(inputs["mass_b1"], np.float32)),
        "mass_w2": np.ascontiguousarray(np.asarray(inputs["mass_w2"], np.float32)),
        "mass_b2": np.ascontiguousarray(np.asarray(inputs["mass_b2"], np.float32)),
        "rff_W": np.ascontiguousarray(np.asarray(inputs["rff_W"], np.float32)),
        "rff_b": np.ascontiguousarray(np.asarray(inputs["rff_b"], np.float32)),
    }
    in_maps = [
        {"U": U[i], "coords": coords[i], **shared} for i in range(B)
    ]
    res = run_bass_kernel_spmd(
        nc, in_maps, list(range(B)), trace=trace, tmpdir=tmpdir
    )
    out = np.stack([res.results[i]["out"].reshape(T, D, R_LR) for i in range(B)])
    return out.astype(np.float32), res


def kernel(**inputs) -> np.ndarray:
    out, _ = run(inputs, trace=False)
    return out
